# revision 1
# baseline (speedup 1.0000x reference)
"""LocallyConnected2d (3x3, stride 1, pad 1) Trainium2 kernel, 8-way spatial-parallel.

out[n,o,h,w] = sum_{c,i,k} weight[o,h,w,c,i,k] * xpad[n,c,h+i,w+k] + bias[o,h,w]

Sharding: output rows h are split 7-per-core across 8 NeuronCores. Each core
streams its private 1/8 weight slice (the dominant ~14.5MB of traffic) exactly
once; x rows are read with a 3-row halo per output row.

Per output row h and padded input column j (0..57), the contraction over
(i, c) = 96 terms is one matmul: lhsT = x column block [96, n=32] (stationary),
rhs = per-pixel weights [96, (pixel, o) <= 96] (moving), accumulated in PSUM
over the 3 columns j = w..w+2 that feed each output pixel w. Bias is folded in
through a K=1 ones-matmul that also initializes the PSUM accumulation group.
"""

import numpy as np

import concourse.bass as bass
import concourse.mybir as mybir
import concourse.tile as tile
from concourse.vector_clock import ScopedClock, VectorClock
from concourse.bass_utils import run_bass_kernel_spmd

N, C, H, W = 32, 32, 56, 56
O = 32
NCORES = 8
R = H // NCORES          # output rows per core
JW = W + 2               # padded input columns
GP = 14                  # pixels per PSUM group (14*32 = 448 <= 512 fp32/bank)
NG = W // GP
KP = 3 * C               # contraction partitions: (i, c)

_patched = False


def _patch_tile_drain():
    """The walrus build in this container rejects >1 sem wait on an InstDrain.
    Move the Tile tail-drain's waits onto one sync-engine nop per processor
    (same-engine in-order issue makes this equivalent), leaving the drain bare.
    """
    global _patched
    if _patched:
        return

    def _drain_and_barrier(self, tick_clock, wait_clock):
        gc = tick_clock.global_clock
        n = len(gc)
        for proc in range(n):
            t = gc[proc]
            if t <= 0:
                continue
            vec = [0] * n
            vec[proc] = t
            nop = self.nc.sync.nop(nofuse=True)
            wait_clock.add_sem_waits(nop.ins, ScopedClock({None: VectorClock(vec)}))
        self.nc.sync.drain()
        self.nc.all_engine_barrier()
        assert self.sems is not None
        popped = self.nc._tile_sem_poison_stack.pop()
        assert popped is self._sem_poison
        self.nc.clear_and_free_semaphores(list(self.sems.allocated().values()))
        self.nc.all_engine_barrier()

    tile.TileContext._drain_and_barrier = _drain_and_barrier
    _patched = True


def _split_multi_waits(nc):
    """This container's walrus accepts at most one semaphore wait per lowered
    instruction (matmul waits land on its single-slot LDWEIGHTS). Hoist all
    but the last wait of every instruction onto same-engine NoOps just before
    it; same-engine in-order issue preserves the wait semantics."""
    ctr = 0
    for fn in nc.m.functions:
        for bb in fn.blocks:
            out = []
            for inst in bb.instructions:
                si = inst.sync_info
                if si is not None and len(si.on_wait) > 1:
                    waits = list(si.on_wait)
                    for w in waits[:-1]:
                        ctr += 1
                        nop = mybir.InstNoOp(
                            name=f"{inst.name}-wsplit-{ctr}",
                            sync_info=mybir.SyncInfo(on_wait=[w], on_update=[]),
                            bass_nofuse=True,
                            engine=inst.engine,
                        )
                        out.append(nop)
                    si.on_wait = [waits[-1]]
                out.append(inst)
            bb.instructions = out
    return ctr


_nc_cache = None


def _build_nc():
    global _nc_cache
    if _nc_cache is not None:
        return _nc_cache
    _patch_tile_drain()
    nc = bass.Bass()
    f32 = mybir.dt.float32
    wt = nc.dram_tensor("wt", [KP, R, JW, 3 * O], f32, kind="ExternalInput")
    xh = nc.dram_tensor("xh", [R + 2, C, JW, N], f32, kind="ExternalInput")
    bc = nc.dram_tensor("bc", [1, R * W * O], f32, kind="ExternalInput")
    out = nc.dram_tensor("out", [N, O, R, W], f32, kind="ExternalOutput")

    with tile.TileContext(nc) as tc:
        with (
            tc.tile_pool(name="singles", bufs=1) as singles,
            tc.tile_pool(name="xp", bufs=2) as xpool,
            tc.tile_pool(name="wp", bufs=2) as wpool,
            tc.tile_pool(name="op", bufs=1) as opool,
            tc.tile_pool(name="ps", bufs=8, space="PSUM") as pspool,
        ):
            ones = singles.tile([1, N], f32)
            nc.vector.memset(ones, 1.0)
            bias_sb = singles.tile([1, R * W * O], f32)
            nc.sync.dma_start(out=bias_sb, in_=bc[:])
            out_sb = opool.tile([N, O * R * W], f32)

            for h in range(R):
                x_t = xpool.tile([KP, JW * N], f32)
                nc.sync.dma_start(
                    out=x_t,
                    in_=xh[h : h + 3].rearrange("r c j n -> (r c) (j n)"),
                )
                w_t = wpool.tile([KP, JW * 3 * O], f32)
                nc.sync.dma_start(
                    out=w_t, in_=wt[:, h].rearrange("p j m -> p (j m)")
                )
                for g in range(NG):
                    wa = g * GP
                    ps = pspool.tile([N, GP * O], f32)
                    nc.tensor.matmul(
                        ps,
                        lhsT=ones,
                        rhs=bias_sb[:, (h * W + wa) * O : (h * W + wa + GP) * O],
                        start=True,
                        stop=False,
                    )
                    for j in range(wa, wa + GP + 2):
                        lo = max(j - 2, wa)
                        hi = min(j, wa + GP - 1)
                        wlo = lo - (j - 2)
                        nwin = hi - lo + 1
                        nc.tensor.matmul(
                            ps[:, (lo - wa) * O : (lo - wa + nwin) * O],
                            lhsT=x_t[:, j * N : (j + 1) * N],
                            rhs=w_t[:, j * 96 + wlo * O : j * 96 + (wlo + nwin) * O],
                            start=False,
                            stop=(j == wa + GP + 1),
                        )
                    # evict psum [n, (w', o)] into out_sb [n, (o, h, w)]
                    src = ps.rearrange("p (w o) -> p o w", o=O)
                    dst = out_sb.rearrange("p (o r w) -> p o r w", o=O, r=R)[
                        :, :, h, wa : wa + GP
                    ]
                    if g % 2 == 0:
                        nc.vector.tensor_copy(out=dst, in_=src)
                    else:
                        nc.scalar.copy(out=dst, in_=src)

            nc.sync.dma_start(out=out[:].rearrange("n o r w -> n (o r w)"), in_=out_sb)
    _split_multi_waits(nc)
    _nc_cache = nc
    return nc


def _pack_core(weight, xp, bias, core):
    h0 = core * R
    Wc = weight[:, h0 : h0 + R]  # [O, R, W, C, 3, 3]
    wtc = np.zeros((3, C, R, JW, 3, O), np.float32)
    for wp in range(3):
        k = 2 - wp
        src = Wc[:, :, :, :, :, k]  # [O, R, W, C, I]
        wtc[:, :, :, 2 - wp : 2 - wp + W, wp, :] = src.transpose(4, 3, 1, 2, 0)
    wtc = np.ascontiguousarray(wtc.reshape(KP, R, JW, 3 * O))
    xhc = np.ascontiguousarray(xp[:, :, h0 : h0 + R + 2, :].transpose(2, 1, 3, 0))
    bcc = np.ascontiguousarray(
        bias[0, :, h0 : h0 + R, :].transpose(1, 2, 0).reshape(1, R * W * O)
    )
    return {"wt": wtc, "xh": xhc, "bc": bcc}


def kernel(x, weight, bias, _want_trace=False):
    x = np.asarray(x, dtype=np.float32)
    weight = np.asarray(weight, dtype=np.float32)
    bias = np.asarray(bias, dtype=np.float32)
    nc = _build_nc()
    xp = np.pad(x, ((0, 0), (0, 0), (1, 1), (1, 1)))
    in_maps = [_pack_core(weight, xp, bias, c) for c in range(NCORES)]
    res = run_bass_kernel_spmd(
        nc, in_maps, core_ids=list(range(NCORES)), trace=_want_trace
    )
    outs = [res.results[i]["out"] for i in range(NCORES)]
    full = np.concatenate(outs, axis=2)
    if _want_trace:
        return full, res
    return full



# revision 5
# speedup vs baseline: 2.0540x; 2.0540x over previous
"""LocallyConnected2d (3x3, stride 1, pad 1) Trainium2 kernel, 8-way spatial-parallel.

out[n,o,h,w] = sum_{c,i,k} weight[o,h,w,c,i,k] * xpad[n,c,h+i,w+k] + bias[o,h,w]

Sharding: output rows h are split 7-per-core across 8 NeuronCores. Each core
streams its private 1/8 weight slice exactly once, in bf16 (host-cast), which
both quarters tensor-engine streaming time vs fp32 and halves HBM traffic.

Per output row h and padded input column j (0..57), the contraction over
(i, c) = 96 terms plus a constant ones-row (97 partitions total) is a matmul:
lhsT = x column block [97, n=32] (stationary; row 96 = 1.0), rhs = per-pixel
weights [97, window <= 3 pixels x O] (moving; row 96 carries the bias for the
window's new pixel, zero elsewhere), accumulated in PSUM over the 3 columns
j = w..w+2 that feed each output pixel w. The first contribution to a pixel
(column j == pixel, kernel k=0, slot 2) is issued as a separate start=True
matmul so no PSUM-init pass is needed; bias rides in on that matmul's row 96.

All input DMAs are issued up front into persistent SBUF tiles (no buffer
rotation stalls); outputs are evicted PSUM->SBUF per 14-pixel group
(vector/scalar alternating) and DMA'd out per row, overlapped with compute.
"""

import numpy as np
import ml_dtypes

import concourse.bass as bass
import concourse.mybir as mybir
import concourse.tile as tile
from concourse.vector_clock import ScopedClock, VectorClock
from concourse.bass_utils import run_bass_kernel_spmd

N, C, H, W = 32, 32, 56, 56
O = 32
NCORES = 8
R = H // NCORES          # output rows per core
JW = W + 2               # padded input columns
GP = 14                  # pixels per PSUM group (14*32 = 448 <= 512 fp32/bank)
NG = W // GP
KP = 3 * C               # contraction partitions: (i, c); +1 ones row below

BF16 = ml_dtypes.bfloat16

_patched = False


def _patch_tile_drain():
    """The walrus build in this container rejects >1 sem wait on an InstDrain.
    Move the Tile tail-drain's waits onto one sync-engine nop per processor
    (same-engine in-order issue makes this equivalent), leaving the drain bare.
    """
    global _patched
    if _patched:
        return

    def _drain_and_barrier(self, tick_clock, wait_clock):
        gc = tick_clock.global_clock
        n = len(gc)
        for proc in range(n):
            t = gc[proc]
            if t <= 0:
                continue
            vec = [0] * n
            vec[proc] = t
            nop = self.nc.sync.nop(nofuse=True)
            wait_clock.add_sem_waits(nop.ins, ScopedClock({None: VectorClock(vec)}))
        self.nc.sync.drain()
        self.nc.all_engine_barrier()
        assert self.sems is not None
        popped = self.nc._tile_sem_poison_stack.pop()
        assert popped is self._sem_poison
        self.nc.clear_and_free_semaphores(list(self.sems.allocated().values()))
        self.nc.all_engine_barrier()

    tile.TileContext._drain_and_barrier = _drain_and_barrier
    _patched = True


def _split_multi_waits(nc):
    """This container's walrus accepts at most one semaphore wait per lowered
    instruction (matmul waits land on its single-slot LDWEIGHTS). Hoist all
    but the last wait of every instruction onto same-engine NoOps just before
    it; same-engine in-order issue preserves the wait semantics."""
    ctr = 0
    for fn in nc.m.functions:
        for bb in fn.blocks:
            out = []
            for inst in bb.instructions:
                si = inst.sync_info
                if si is not None and len(si.on_wait) > 1:
                    waits = list(si.on_wait)
                    for w in waits[:-1]:
                        ctr += 1
                        nop = mybir.InstNoOp(
                            name=f"{inst.name}-wsplit-{ctr}",
                            sync_info=mybir.SyncInfo(on_wait=[w], on_update=[]),
                            bass_nofuse=True,
                            engine=inst.engine,
                        )
                        out.append(nop)
                    si.on_wait = [waits[-1]]
                out.append(inst)
            bb.instructions = out
    return ctr


_nc_cache = None


def _build_nc():
    global _nc_cache
    if _nc_cache is not None:
        return _nc_cache
    _patch_tile_drain()
    nc = bass.Bass()
    f32 = mybir.dt.float32
    bf16 = mybir.dt.bfloat16
    wt = nc.dram_tensor("wt", [KP + 1, R, JW, 3 * O], bf16, kind="ExternalInput")
    xh = nc.dram_tensor("xh", [R + 2, C, JW, N], bf16, kind="ExternalInput")
    on = nc.dram_tensor("on", [1, JW * N], bf16, kind="ExternalInput")
    out = nc.dram_tensor("out", [N, R, O, W], f32, kind="ExternalOutput")

    with tile.TileContext(nc) as tc:
        with (
            tc.tile_pool(name="singles", bufs=1) as singles,
            tc.tile_pool(name="ps", bufs=8, space="PSUM") as pspool,
        ):
            w_t = [
                singles.tile([KP + 1, JW * 3 * O], bf16, name=f"w{h}")
                for h in range(R)
            ]
            x_t = [
                singles.tile([KP + 1, JW * N], bf16, name=f"x{h}") for h in range(R)
            ]
            o_t = [singles.tile([N, O * W], f32, name=f"o{h}") for h in range(R)]

            # All input DMAs up front: x (+ones row) on the scalar queue,
            # weights on the sync queue, interleaved so row h's operands
            # arrive roughly in compute order.
            # NB: DMAs with an odd partition count get pinned to a single DMA
            # engine (observed on HW: 97-partition transfers all landed on one
            # engine at 1/16th bandwidth). Issue the 96 weight rows and the
            # bias/ones row as separate DMAs so the big one round-robins.
            for h in range(R):
                nc.scalar.dma_start(
                    out=x_t[h][: KP, :],
                    in_=xh[h : h + 3].rearrange("r c j n -> (r c) (j n)"),
                )
                nc.scalar.dma_start(out=x_t[h][KP : KP + 1, :], in_=on[:])
                nc.sync.dma_start(
                    out=w_t[h][: KP, :],
                    in_=wt[: KP, h].rearrange("p j m -> p (j m)"),
                )
                nc.sync.dma_start(
                    out=w_t[h][KP : KP + 1, :],
                    in_=wt[KP :, h].rearrange("p j m -> p (j m)"),
                )

            for h in range(R):
                for g in range(NG):
                    wa = g * GP
                    last_j = wa + GP + 1
                    ps = pspool.tile([N, GP * O], f32)
                    # start=True on HW zeroes the WHOLE PSUM bank, then writes
                    # the addressed columns (verified empirically). So one
                    # start=True on the group's first matmul initializes the
                    # group; every later matmul accumulates — including each
                    # pixel's first touch (slot 2 at column j == pixel), which
                    # also carries bias via weight row 96 x ones row 96.
                    for j in range(wa, wa + GP + 2):
                        lo = max(j - 2, wa)
                        hi = min(j, wa + GP - 1)
                        base = j * 3 * O
                        wlo = lo - j + 2
                        nwin = hi - lo + 1
                        nc.tensor.matmul(
                            ps[:, (lo - wa) * O : (hi + 1 - wa) * O],
                            lhsT=x_t[h][:, j * N : (j + 1) * N],
                            rhs=w_t[h][:, base + wlo * O : base + (wlo + nwin) * O],
                            start=(j == wa),
                            stop=(j == last_j),
                        )
                    # evict psum [n, (w', o)] into o_t[h] [n, (o, w)]
                    src = ps.rearrange("p (w o) -> p o w", o=O)
                    dst = o_t[h].rearrange("p (o w) -> p o w", o=O)[:, :, wa : wa + GP]
                    if g % 2 == 0:
                        nc.vector.tensor_copy(out=dst, in_=src)
                    else:
                        nc.scalar.copy(out=dst, in_=src)
                nc.sync.dma_start(
                    out=out[:, h].rearrange("n o w -> n (o w)"), in_=o_t[h]
                )
    _split_multi_waits(nc)
    _nc_cache = nc
    return nc


def _pack_core(weight, xp, bias, core):
    h0 = core * R
    Wc = weight[:, h0 : h0 + R]  # [O, R, W, C, 3, 3]
    wtc = np.zeros((KP + 1, R, JW, 3 * O), np.float32)
    w96 = wtc[:KP].reshape(3, C, R, JW, 3, O)
    for wp in range(3):
        k = 2 - wp
        src = Wc[:, :, :, :, :, k]  # [O, R, W, C, I]
        w96[:, :, :, 2 - wp : 2 - wp + W, wp, :] = src.transpose(4, 3, 1, 2, 0)
    # bias rides in row 96, slot 2 (the start=True matmul for pixel j at
    # column j), multiplied by the constant 1.0 in x's row 96.
    brow = wtc[KP].reshape(R, JW, 3, O)
    brow[:, :W, 2, :] = bias[0, :, h0 : h0 + R, :].transpose(1, 2, 0)
    xhc = xp[:, :, h0 : h0 + R + 2, :].transpose(2, 1, 3, 0)
    return {
        "wt": np.ascontiguousarray(wtc).astype(BF16),
        "xh": np.ascontiguousarray(xhc).astype(BF16),
        "on": np.ones((1, JW * N), BF16),
    }


def kernel(x, weight, bias, _want_trace=False):
    x = np.asarray(x, dtype=np.float32)
    weight = np.asarray(weight, dtype=np.float32)
    bias = np.asarray(bias, dtype=np.float32)
    nc = _build_nc()
    xp = np.pad(x, ((0, 0), (0, 0), (1, 1), (1, 1)))
    in_maps = [_pack_core(weight, xp, bias, c) for c in range(NCORES)]
    res = run_bass_kernel_spmd(
        nc, in_maps, core_ids=list(range(NCORES)), trace=_want_trace
    )
    # per-core out is [N, R, O, W]; assemble to [N, O, H, W]
    outs = [res.results[i]["out"].transpose(0, 2, 1, 3) for i in range(NCORES)]
    full = np.concatenate(outs, axis=2)
    if _want_trace:
        return full, res
    return full
